# revision 60
# baseline (speedup 1.0000x reference)
"""Trainium2 Bass kernel for the attention+LN+MLP block (nn_Attention_84310208020626).

Reference computation (per batch b):
    q = x_b @ Wq.T ; k = x_b @ Wk.T ; v = x_b @ Wv.T          (S=2048, D=512)
    attn = softmax(q k^T / sqrt(512))
    res  = attn @ v
    h    = LayerNorm(res) * ln_g + ln_b
    out  = relu(h @ W1.T + b1) @ W2.T + b2

Sharding: 8 cores = 4 batches x 2 sequence halves. Every core computes its
batch's full K/V (recompute, no collectives) and runs attention + LN + MLP
for its own 1024 query rows.

Device layout: activations are feature-major [feature, seq] so that every
GEMM contracts over the partition dimension without transposes:
    GT[d',s]     = A-stationary GEMM over xT, A = Wq^T Wk precomputed on host
                   (scores = q k^T = (x A) x^T, so no separate Q/K GEMMs)
    scoresT[t,s] = xT-stationary GEMM, rhs = GT     -> exp -> expT (bf16)
    Z[d,s]       = xTM-stationary GEMM over expT (t-contraction)
    resU[e,s]    = Wv-stationary GEMM, rhs = Z  (softmax denom NOT applied)
LayerNorm over e (partition dim): column sums of resU / resU^2 via a DVE
pair-tree + one ones-matmul each. The softmax division folds into LN via
scale invariance; the eps term eps*sums^2 is dropped (varU is ~1e6 x larger,
relative effect ~1e-5, far below the 2e-2 gate) so the softmax denominator
is never computed at all. The whole LN is folded into the MLP1 GEMM epilogue:
    h1 = relu( (G1 @ res)*rstd[s] - murstd[s]*r1[f] + (W1@ln_b + b1)[f] )
with G1 = W1*diag(ln_g), r1 = G1 row sums, both precomputed on HOST.
Per-column stats are broadcast across partitions with a K=1 ones matmul.
All GEMM operands are bf16 (fp32 PSUM accumulation); LN stats math is fp32.
Engine split: PE matmuls; ACT exp + row smalls; DVE res copies, stats tree,
MLP1 epilogue; Pool (gpsimd) qt/z psum copies + out bias.
"""

import ml_dtypes
import numpy as np

import concourse.bass as bass
import concourse.mybir as mybir
import concourse.tile as tile
from concourse import bacc
from concourse.bass_utils import run_bass_kernel_spmd

S, B, D = 2048, 4, 512
N_CORES = 8
SQ = 1024          # query rows per core
SBLK = 512         # s-block (pipeline granularity)
NBLK = SQ // SBLK  # 2
ND = D // 128      # 4 chunks of the feature dims
NT = S // 128      # 16 t-chunks
NTT = S // 512     # 4 t-tiles of 512 for KT GEMM
VAR_FLOOR = 1e-3   # replaces eps*sums^2 (varU ~ 1e6, see module docstring)
SCALE = 1.0 / float(np.sqrt(512.0))

F32 = mybir.dt.float32
F32R = mybir.dt.float32r
BF16 = mybir.dt.bfloat16
AF = mybir.ActivationFunctionType
ALU = mybir.AluOpType


def _emit(nc, tc, n_iters=1):
    xT = nc.tensor_by_name["xT"].ap()       # (512, 2048) bf16, q-half first
    xTM = nc.tensor_by_name["xTM"].ap()     # (2048, 512) bf16, same t order
    A_qk = nc.tensor_by_name["A_qk"].ap()   # (512, 512) = Wq.T @ Wk  (d, d')
    WvT = nc.tensor_by_name["WvT"].ap()
    W1T = nc.tensor_by_name["W1T"].ap()     # (512, 512) = G1.T  (e, f), host-folded
    W2T = nc.tensor_by_name["W2T"].ap()
    r1 = nc.tensor_by_name["r1"].ap()       # (512,) = W1 @ ln_g
    w1bb1 = nc.tensor_by_name["w1bb1"].ap()  # (512,) = W1 @ ln_b + b1
    b2 = nc.tensor_by_name["b2"].ap()
    outT = nc.tensor_by_name["outT"].ap()   # (512, 1024) bf16 out

    # ---------------- SBUF tiles ----------------
    from contextlib import ExitStack
    ctx = ExitStack()
    consts = ctx.enter_context(tc.tile_pool(name="consts", bufs=1))
    big = ctx.enter_context(tc.tile_pool(name="big", bufs=1))
    qt_pool = ctx.enter_context(tc.tile_pool(name="qt", bufs=2))
    exp_pool = ctx.enter_context(tc.tile_pool(name="expp", bufs=2))
    res_pool = ctx.enter_context(tc.tile_pool(name="resp", bufs=2))
    h1_pool = ctx.enter_context(tc.tile_pool(name="h1p", bufs=2))
    out_pool = ctx.enter_context(tc.tile_pool(name="outp", bufs=2))
    sq_pool = ctx.enter_context(tc.tile_pool(name="sqp", bufs=8))
    z_pool = ctx.enter_context(tc.tile_pool(name="zp", bufs=2))
    row_pool = ctx.enter_context(tc.tile_pool(name="rowp", bufs=2))
    bc_pool = ctx.enter_context(tc.tile_pool(name="bcp", bufs=2))

    mm_psum = ctx.enter_context(tc.tile_pool(name="mmps", bufs=8, space="PSUM"))

    # constants / weights (tiles only — the DMAs are emitted inside the first
    # _emit_iter in global priority order on a single queue, because the DMA
    # fabric is one serial ~358 GB/s pipe and arrival order is what counts)
    a_sb = consts.tile([128, ND, D], BF16)    # (p, dc, d')
    wv_sb = consts.tile([128, ND, D], BF16)
    w1_sb = consts.tile([128, ND, D], BF16)   # G1.T, host-folded
    w2_sb = consts.tile([128, ND, D], BF16)
    b2_sb = consts.tile([128, ND], F32)
    r1_sb = consts.tile([128, ND], F32)
    w1bb1_sb = consts.tile([128, ND], F32)

    ones128 = nc.tensor_by_name["ones128"].ap()  # (128,) of 1.0
    ones_col_b = consts.tile([128, 1], BF16)   # stationary for column sums
    nc.vector.memset(ones_col_b, 1.0)
    warm_sb = consts.tile([128, 512], BF16)    # PE p-state warmup fodder
    nc.vector.memset(warm_sb, 0.0)
    ones_row = consts.tile([1, 128], F32R)      # stationary for partition broadcast

    def emit_const_dmas():
        # column-chunked so GT's first psum group starts after 1/4 of A
        ar = A_qk.rearrange("(dc p) e -> p dc e", p=128)
        for ec in range(ND):
            nc.sync.dma_start(out=a_sb[:, :, ec * 128:(ec + 1) * 128],
                              in_=ar[:, :, ec * 128:(ec + 1) * 128])

    def emit_const_dmas_late():
        wvr = WvT.rearrange("(dc p) e -> p dc e", p=128)
        nc.sync.dma_start(out=wv_sb[:, :, :], in_=wvr[:, :, :])
        for w_sb, w_dram in ((w1_sb, W1T), (w2_sb, W2T)):
            wr = w_dram.rearrange("(dc p) e -> p dc e", p=128)
            nc.sync.dma_start(out=w_sb[:, :, :], in_=wr[:, :, :])
        for v_sb, v_dram in ((b2_sb, b2), (r1_sb, r1), (w1bb1_sb, w1bb1)):
            nc.sync.dma_start(out=v_sb[:, :],
                              in_=v_dram.rearrange("(c p) -> p c", p=128))
        nc.sync.dma_start(out=ones_row[:, :],
                          in_=ones128.bitcast(F32R).rearrange("(c p) -> c p", c=1))

    for _iter in range(n_iters):
        _emit_iter(nc, tc, xT, xTM, outT, big, qt_pool, exp_pool, res_pool, h1_pool,
                   out_pool, sq_pool, z_pool, row_pool, bc_pool, mm_psum,
                   a_sb, wv_sb, w1_sb, w2_sb, b2_sb,
                   ones_col_b, ones_row, r1_sb, w1bb1_sb,
                   emit_const_dmas if _iter == 0 else None,
                   emit_const_dmas_late if _iter == 0 else None,
                   warm_sb if _iter == 0 else None)

    ctx.close()


def _emit_iter(nc, tc, xT, xTM, outT, big, qt_pool, exp_pool, res_pool, h1_pool,
               out_pool, sq_pool, z_pool, row_pool, bc_pool, mm_psum,
               a_sb, wv_sb, w1_sb, w2_sb, b2_sb,
               ones_col_b, ones_row, r1_sb, w1bb1_sb,
               const_dmas=None, const_dmas_late=None, warm_sb=None):
    # All input DMAs ride one queue in consumption-priority order (the DMA
    # fabric is a single serial ~358 GB/s pipe): x chunk0, A, rest of x,
    # xtm, then weights and smalls. The PE warmup covers the head latency.
    if warm_sb is not None:
        # PE p-state warmup: dummy matmuls bridge the input-DMA head so the
        # tensor engine clock is fully ramped when GT's operands land.
        for wi in range(7):
            wps = mm_psum.tile([128, 512], F32, tag="mm", name=f"w{wi}")
            nc.tensor.matmul(wps[:, :], warm_sb[:, 0:128], warm_sb[:, :],
                             start=True, stop=True)
    x_sb = big.tile([128, ND, S], BF16, tag="x", name="x_sb")
    xr = xT.rearrange("(dc p) t -> p dc t", p=128)
    nc.sync.dma_start(out=x_sb[:, :, 0:512], in_=xr[:, :, 0:512])
    if const_dmas is not None:
        const_dmas()
    for tt in range(1, NTT):
        nc.sync.dma_start(out=x_sb[:, :, tt * 512:(tt + 1) * 512],
                          in_=xr[:, :, tt * 512:(tt + 1) * 512])

    # x in t-major layout: stationary of the Z = x^T @ exp GEMM
    xtm_sb = big.tile([128, NT, D], BF16, tag="v", name="xtm_sb")
    xmr = xTM.rearrange("(tc p) d -> p tc d", p=128)
    for g in range(4):
        nc.sync.dma_start(out=xtm_sb[:, 4 * g:4 * (g + 1), :],
                          in_=xmr[:, 4 * g:4 * (g + 1), :])
    if const_dmas_late is not None:
        const_dmas_late()

    # ------- GT = A-stationary GEMM (G = x @ A; scores = G @ x^T) -------
    qt_tiles = []
    for sb in range(NBLK):
        s0 = sb * SBLK
        qt_sb = qt_pool.tile([128, ND, SBLK], BF16, tag="qt")
        for ec in range(ND):
            qps = mm_psum.tile([128, 512], F32, tag="mm")
            for dc in range(ND):
                nc.tensor.matmul(
                    qps[:, :],
                    a_sb[:, dc, ec * 128:(ec + 1) * 128],
                    x_sb[:, dc, s0:s0 + SBLK],
                    start=(dc == 0), stop=(dc == ND - 1),
                )
            nc.vector.tensor_copy(out=qt_sb[:, ec, :], in_=qps[:, :])
        qt_tiles.append(qt_sb)

    # ---------------- per s-block pipeline (software-pipelined emission) ----
    exp_tiles = [None] * NBLK
    res_tiles = [None] * NBLK
    sq_tiles = [None] * NBLK
    rows2_tiles = [None] * NBLK

    def emit_scores(sb):
        qt_sb = qt_tiles[sb]
        exp_sb = exp_pool.tile([128, NT, SBLK], BF16, tag="exp", name=f"exp{sb}")
        for tc_i in range(NT):
            sps = mm_psum.tile([128, 512], F32, tag="mm")
            for dc in range(ND):
                nc.tensor.matmul(
                    sps[:, :],
                    x_sb[:, dc, tc_i * 128:(tc_i + 1) * 128],
                    qt_sb[:, dc, :],
                    start=(dc == 0), stop=(dc == ND - 1),
                )
            nc.scalar.activation(out=exp_sb[:, tc_i, :], in_=sps[:, :],
                                 func=AF.Exp, scale=SCALE)
        exp_tiles[sb] = exp_sb

    res_ps_tiles = [None] * NBLK

    def emit_res(sb):
        exp_sb = exp_tiles[sb]
        # Z[d, s] = sum_t x[t,d] * exp[t,s]   (x t-major stationary)
        z_sb = z_pool.tile([128, ND, SBLK], BF16, tag="z", name=f"z{sb}")
        for dc in range(ND):
            zps = mm_psum.tile([128, 512], F32, tag="mm")
            for tc_i in range(NT):
                nc.tensor.matmul(
                    zps[:, :],
                    xtm_sb[:, tc_i, dc * 128:(dc + 1) * 128],
                    exp_sb[:, tc_i, :],
                    start=(tc_i == 0), stop=(tc_i == NT - 1),
                )
            nc.scalar.copy(out=z_sb[:, dc, :], in_=zps[:, :])
        # resU[e, s] = Wv @ Z; bf16 copies on ACT (latency-critical for the
        # stats matmuls), squares on DVE from the psums
        res_sb = res_pool.tile([128, ND, SBLK], BF16, tag="res", name=f"res{sb}")
        sq_l = []
        for ec in range(ND):
            rps = mm_psum.tile([128, 512], F32, tag="mm", name=f"r{sb}_{ec}")
            for dc in range(ND):
                nc.tensor.matmul(
                    rps[:, :],
                    wv_sb[:, dc, ec * 128:(ec + 1) * 128],
                    z_sb[:, dc, :],
                    start=(dc == 0), stop=(dc == ND - 1),
                )
            nc.scalar.copy(out=res_sb[:, ec, :], in_=rps[:, :])
            sq_t = sq_pool.tile([128, SBLK], BF16, tag="sq", name=f"s{sb}_{ec}")
            nc.vector.tensor_mul(out=sq_t[:, :], in0=res_sb[:, ec, :],
                                 in1=res_sb[:, ec, :])
            sq_l.append(sq_t)
        res_tiles[sb] = res_sb
        sq_tiles[sb] = sq_l

    def emit_stats_mm(sb):
        # PE partition reduction (accumulating ones-matmuls over the 4 e-chunks
        # of res / res^2) + the scalar row-stat chain:
        #   muU = sumE/512 ; varU = sumSq/512 - muU^2
        #   rstd = 1/sqrt(varU + VAR_FLOOR) ; murstd = muU*rstd
        res_sb = res_tiles[sb]
        sq_l = sq_tiles[sb]
        sume_ps = mm_psum.tile([1, 512], F32, tag="mm")
        for ec in range(ND):
            nc.tensor.matmul(sume_ps[:, :], ones_col_b[:, :], res_sb[:, ec, :],
                             start=(ec == 0), stop=(ec == ND - 1))
        sumsq_ps = mm_psum.tile([1, 512], F32, tag="mm")
        for ec in range(ND):
            nc.tensor.matmul(sumsq_ps[:, :], ones_col_b[:, :], sq_l[ec][:, :],
                             start=(ec == 0), stop=(ec == ND - 1))

        # row chain on DVE (its queue is clear at both uses); sqrt on ACT
        rows = row_pool.tile([1, 3, SBLK], F32, tag="rows", name=f"rows{sb}")
        rows2 = row_pool.tile([1, 2, SBLK], F32R, tag="rows2", name=f"rows2{sb}")
        nc.scalar.mul(out=rows[:, 0, :], in_=sume_ps[:, :], mul=-1.0 / D)    # -muU
        nc.vector.tensor_scalar(out=rows[:, 1, :], in0=sumsq_ps[:, :],
                                scalar1=1.0 / D, scalar2=VAR_FLOOR,
                                op0=ALU.mult, op1=ALU.add)  # msq + floor
        nc.vector.tensor_mul(out=rows[:, 2, :], in0=rows[:, 0, :], in1=rows[:, 0, :])
        nc.vector.tensor_sub(out=rows[:, 1, :], in0=rows[:, 1, :], in1=rows[:, 2, :])
        nc.scalar.activation(out=rows2[:, 0, :], in_=rows[:, 1, :],
                             func=AF.Abs_reciprocal_sqrt)                    # rstd
        nc.vector.tensor_mul(out=rows2[:, 1, :], in0=rows[:, 0, :],
                             in1=rows2[:, 0, :])                             # -murstd
        rows2_tiles[sb] = rows2

    p_tiles = [None] * NBLK
    h1_tiles = [None] * NBLK

    def emit_p(sb):
        # P = G1 @ res (independent of the LN stats chain)
        res_sb = res_tiles[sb]
        p_ps = []
        for fc in range(ND):
            hps = mm_psum.tile([128, 512], F32, tag="mm", name=f"p{sb}_{fc}")
            for ec in range(ND):
                nc.tensor.matmul(
                    hps[:, :],
                    w1_sb[:, ec, fc * 128:(fc + 1) * 128],
                    res_sb[:, ec, :],
                    start=(ec == 0), stop=(ec == ND - 1),
                )
            p_ps.append(hps)
        p_tiles[sb] = p_ps

    def emit_bc_epi(sb):
        rows2 = rows2_tiles[sb]
        p_ps = p_tiles[sb]

        # broadcast [rstd; -murstd] across 128 partitions via K=1 matmul;
        # psum->sbuf copies on ACT (free in the late phase)
        bc_sb = bc_pool.tile([128, 2, SBLK], F32, tag="bc_sb")
        for j in range(2):
            bc_ps = mm_psum.tile([128, 512], F32, tag="mm")
            nc.tensor.matmul(
                bc_ps[:, :], ones_row[:, :],
                rows2[:, j, :], start=True, stop=True,
            )
            nc.scalar.copy(out=bc_sb[:, j, :], in_=bc_ps[:, :])

        # fused MLP1 + LayerNorm epilogue:
        #   h1 = relu( P*rstd[s] + u_fc ),  u_fc = -murstd*r1[f] + w1bb1[f]
        # u is P-independent (one tensor_scalar per fc, off the critical
        # path); the P-dependent mul+add run on DVE for fc0/2 and on Pool
        # (via an ACT psum->sbuf copy) for fc1/3; relu on ACT.
        h1_sb = h1_pool.tile([128, ND, SBLK], BF16, tag="h1", name=f"h1_{sb}")
        for fc in range(ND):
            u_sb = sq_pool.tile([128, SBLK], F32, tag="sqt", name=f"u{sb}_{fc}")
            nc.vector.tensor_scalar(
                out=u_sb[:, :], in0=bc_sb[:, 1, :],
                scalar1=r1_sb[:, fc:fc + 1], scalar2=w1bb1_sb[:, fc:fc + 1],
                op0=ALU.mult, op1=ALU.add)
            t_sb = sq_pool.tile([128, SBLK], F32, tag="sqt2")
            if fc % 2 == 0:
                nc.vector.tensor_mul(out=t_sb[:, :], in0=p_ps[fc][:, :],
                                     in1=bc_sb[:, 0, :])
                nc.vector.tensor_add(out=t_sb[:, :], in0=t_sb[:, :],
                                     in1=u_sb[:, :])
            else:
                psb = sq_pool.tile([128, SBLK], F32, tag="sqt3")
                nc.scalar.copy(out=psb[:, :], in_=p_ps[fc][:, :])
                nc.gpsimd.tensor_mul(out=t_sb[:, :], in0=psb[:, :],
                                     in1=bc_sb[:, 0, :])
                nc.gpsimd.tensor_add(out=t_sb[:, :], in0=t_sb[:, :],
                                     in1=u_sb[:, :])
            nc.scalar.activation(out=h1_sb[:, fc, :], in_=t_sb[:, :],
                                 func=AF.Relu)
        h1_tiles[sb] = h1_sb

    def emit_mlp2(sb):
        # fc-major matmul order so each h1 chunk is consumed as soon as its
        # epilogue lands, instead of waiting for the full h1 tile.
        s0 = sb * SBLK
        h1_sb = h1_tiles[sb]
        o_sb = out_pool.tile([128, ND, SBLK], BF16, tag="o")
        outr = outT[:, s0:s0 + SBLK].rearrange("(gc p) s -> p gc s", p=128)
        ops_l = [mm_psum.tile([128, 512], F32, tag="mm", name=f"o{sb}_{g}")
                 for g in range(ND)]
        for fc in range(ND):
            for gc in range(ND):
                nc.tensor.matmul(
                    ops_l[gc][:, :],
                    w2_sb[:, fc, gc * 128:(gc + 1) * 128],
                    h1_sb[:, fc, :],
                    start=(fc == 0), stop=(fc == ND - 1),
                )
        for gc in range(ND):
            if gc % 2 == 0:
                nc.vector.tensor_scalar_add(out=o_sb[:, gc, :],
                                            in0=ops_l[gc][:, :],
                                            scalar1=b2_sb[:, gc:gc + 1])
            else:
                nc.scalar.activation(out=o_sb[:, gc, :], in_=ops_l[gc][:, :],
                                     func=AF.Identity,
                                     bias=b2_sb[:, gc:gc + 1])
            eng = nc.sync if gc % 2 == 0 else nc.gpsimd
            eng.dma_start(out=outr[:, gc, :], in_=o_sb[:, gc, :])

    emit_scores(0)
    emit_res(0)
    emit_scores(1)
    emit_stats_mm(0)
    emit_res(1)
    emit_p(0)
    emit_stats_mm(1)
    emit_bc_epi(0)
    emit_p(1)
    emit_bc_epi(1)
    emit_mlp2(0)
    emit_mlp2(1)


def build_nc(n_iters=1):
    nc = bacc.Bacc("TRN2", target_bir_lowering=False, debug=False)
    nc.tensor_by_name = {}

    def dram(name, shape, kind):
        t = nc.dram_tensor(name, shape, F32, kind=kind)
        nc.tensor_by_name[name] = t
        return t

    def dram_bf(name, shape, kind):
        t = nc.dram_tensor(name, shape, BF16, kind=kind)
        nc.tensor_by_name[name] = t
        return t

    dram_bf("xT", [D, S], "ExternalInput")
    dram_bf("xTM", [S, D], "ExternalInput")
    for nm in ("A_qk", "WvT", "W1T", "W2T"):
        dram_bf(nm, [D, D], "ExternalInput")
    for nm in ("b2", "r1", "w1bb1"):
        dram(nm, [D], "ExternalInput")
    dram("ones128", [128], "ExternalInput")
    dram_bf("outT", [D, SQ], "ExternalOutput")

    with tile.TileContext(nc) as tc:
        _emit(nc, tc, n_iters=n_iters)
    nc.compile()
    return nc


_CACHED_NC = None


def _get_nc():
    global _CACHED_NC
    if _CACHED_NC is None:
        _CACHED_NC = build_nc()
    return _CACHED_NC


def make_in_maps(x, Wq, Wk, Wv, ln_g, ln_b, W1, b1, W2, b2):
    BF = ml_dtypes.bfloat16
    x = np.asarray(x, dtype=np.float32)
    W1 = np.asarray(W1, np.float32)
    ln_g = np.asarray(ln_g, np.float32)
    ln_b = np.asarray(ln_b, np.float32)
    A_qk = np.asarray(Wq, np.float32).T @ np.asarray(Wk, np.float32)
    G1T = W1.T * ln_g[:, None]           # (e, f), LN gamma folded into MLP1
    r1 = W1 @ ln_g                       # (f,)
    w1bb1 = W1 @ ln_b + np.asarray(b1, np.float32)
    shared = {
        "A_qk": np.ascontiguousarray(A_qk.astype(BF)),
        "WvT": np.ascontiguousarray(np.asarray(Wv, np.float32).T.astype(BF)),
        "W1T": np.ascontiguousarray(G1T.astype(BF)),
        "W2T": np.ascontiguousarray(np.asarray(W2, np.float32).T.astype(BF)),
        "b2": np.asarray(b2, np.float32),
        "r1": r1,
        "w1bb1": w1bb1,
        "ones128": np.ones(128, np.float32),
    }
    in_maps = []
    for c in range(N_CORES):
        b, h = divmod(c, 2)
        xT = x[:, b, :].T  # (512, 2048)
        q = xT[:, h * SQ:(h + 1) * SQ]
        o = xT[:, (1 - h) * SQ:(2 - h) * SQ]
        xp = np.concatenate([q, o], axis=1)  # (512, 2048), q-half first
        in_maps.append({"xT": np.ascontiguousarray(xp.astype(BF)),
                        "xTM": np.ascontiguousarray(xp.T.astype(BF)),
                        **shared})
    return in_maps


def kernel(x, Wq, Wk, Wv, ln_g, ln_b, W1, b1, W2, b2):
    nc = _get_nc()
    in_maps = make_in_maps(x, Wq, Wk, Wv, ln_g, ln_b, W1, b1, W2, b2)
    res = run_bass_kernel_spmd(nc, in_maps, list(range(N_CORES)))
    out = np.empty((S, B, D), dtype=np.float32)
    for c in range(N_CORES):
        b, h = divmod(c, 2)
        out[h * SQ:(h + 1) * SQ, b, :] = res.results[c]["outT"].T.astype(np.float32)
    return out
